# revision 1
# baseline (speedup 1.0000x reference)
"""Windowed dense attention (Swin-style, dynamic relative-position bias MLP).

Contract: kernel(**inputs) takes the FULL unsharded inputs (see shapes below)
and returns the FULL output (B, H*W, C) float32.

Strategy (per spec sharding_hint): pure data parallel over batch B=8 across the
8 NeuronCores. Each core processes one batch image's 1024 windows end-to-end
(qkv projection, windowed attention with relative-position bias + padding mask,
softmax, AV, output projection). The tiny bias-MLP weights and the 64x64
relative-position tables are replicated to every core.

Hardcoded problem shape: x (8, 250, 250, 28), G=8 window, 4 heads, head_dim 7.

Implementation: the per-batch computation is expressed in jax and compiled for
the NeuronCores through the PJRT backend (one pmap over the 8 cores). If the
device path is unavailable at runtime, a numpy fallback computes the same
values on host so the function always returns correct results.
"""

import numpy as np

NEG_INF = -1000000.0
G = 8
DIM = 28
NH = 4
HD = DIM // NH
B, H, W = 8, 250, 250
PAD = (-H) % G          # 6
HD_PAD, WD_PAD = H + PAD, W + PAD   # 256, 256
NWIN = (HD_PAD // G) * (WD_PAD // G)  # 1024
GG = G * G              # 64
SCALE = HD ** (-0.5)

_jit_cache = {}


def _rel_pos_index():
    coords = np.stack(np.meshgrid(np.arange(G), np.arange(G), indexing="ij"))
    cf = coords.reshape(2, -1)
    rel = cf[:, :, None] - cf[:, None, :]
    rel = rel.transpose(1, 2, 0).astype(np.int64)
    rel[:, :, 0] += G - 1
    rel[:, :, 1] += G - 1
    rel[:, :, 0] *= 2 * G - 1
    return rel.sum(-1).astype(np.int32)  # (GG, GG)


def _biases():
    ph = np.arange(1 - G, G)
    b = np.stack(np.meshgrid(ph, ph, indexing="ij")).reshape(2, -1).T
    return b.astype(np.float32)  # (225, 2)


def _layernorm_np(x, g, b, eps=1e-5):
    m = x.mean(-1, keepdims=True)
    v = ((x - m) ** 2).mean(-1, keepdims=True)
    return (x - m) / np.sqrt(v + eps) * g + b


def _pos_bias_np(pos_proj_w, pos_proj_b, ln1_g, ln1_b, fc1_w, fc1_b,
                 ln2_g, ln2_b, fc2_w, fc2_b, ln3_g, ln3_b, fc3_w, fc3_b):
    """Tiny MLP over the 225 relative offsets -> (NH, GG, GG) bias table.

    225x1 matmuls are negligible; computing this on host (float32, identical
    op order to the reference) and replicating the result to all cores is the
    'replicate the shared 64x64 relative-position bias' part of the sharding.
    """
    t = _biases() @ np.asarray(pos_proj_w).T + np.asarray(pos_proj_b)
    t = _layernorm_np(t, np.asarray(ln1_g), np.asarray(ln1_b))
    t = np.maximum(t, 0.0) @ np.asarray(fc1_w).T + np.asarray(fc1_b)
    t = _layernorm_np(t, np.asarray(ln2_g), np.asarray(ln2_b))
    t = np.maximum(t, 0.0) @ np.asarray(fc2_w).T + np.asarray(fc2_b)
    t = _layernorm_np(t, np.asarray(ln3_g), np.asarray(ln3_b))
    pos = np.maximum(t, 0.0) @ np.asarray(fc3_w).T + np.asarray(fc3_b)  # (225, NH)
    rpb = pos[_rel_pos_index()]              # (GG, GG, NH)
    return np.ascontiguousarray(rpb.transpose(2, 0, 1)).astype(np.float32)


def _attn_mask_np():
    """Additive key-padding mask per window: (NWIN, 1, GG)."""
    m2d = np.zeros((HD_PAD, WD_PAD), np.float32)
    m2d[-PAD:, :] = -1.0
    m2d[:, -PAD:] = -1.0
    nh = HD_PAD // G
    nw = WD_PAD // G
    mw = m2d.reshape(nh, G, nw, G).transpose(0, 2, 1, 3).reshape(NWIN, GG)
    return np.where(mw[:, None, :] < 0, NEG_INF, 0.0).astype(np.float32)


def _window_partition_np(x):
    """(B, H, W, C) -> padded, windowed (B, NWIN, GG, C)."""
    xp = np.zeros((B, HD_PAD, WD_PAD, DIM), np.float32)
    xp[:, :H, :W, :] = x
    nh = HD_PAD // G
    nw = WD_PAD // G
    xw = xp.reshape(B, nh, G, nw, G, DIM).transpose(0, 1, 3, 2, 4, 5)
    return np.ascontiguousarray(xw.reshape(B, NWIN, GG, DIM))


def _window_merge_np(ow):
    """(B, NWIN, GG, C) -> cropped (B, H*W, C)."""
    nh = HD_PAD // G
    nw = WD_PAD // G
    o = ow.reshape(B, nh, nw, G, G, DIM).transpose(0, 1, 3, 2, 4, 5)
    o = o.reshape(B, HD_PAD, WD_PAD, DIM)[:, :H, :W, :]
    return np.ascontiguousarray(o.reshape(B, H * W, DIM))


def _device_fn():
    """Build (once) the pmapped per-core window-attention function."""
    if "fn" in _jit_cache:
        return _jit_cache["fn"]
    import jax
    import jax.numpy as jnp

    devs = jax.devices()[:8]
    if len(devs) < 8:
        raise RuntimeError("need 8 cores")

    def core_fn(xw, qkv_w, qkv_b, proj_w, proj_b, rpb, mask):
        # xw: (NWIN, GG, C) one batch image's windows on this core
        qkv = xw @ qkv_w.T + qkv_b                       # (NWIN, GG, 3C)
        qkv = qkv.reshape(NWIN, GG, 3, NH, HD).transpose(2, 0, 3, 1, 4)
        q, k, v = qkv[0], qkv[1], qkv[2]                 # (NWIN, NH, GG, HD)
        attn = jnp.einsum("whnd,whmd->whnm", q * SCALE, k)
        attn = attn + rpb[None] + mask[:, None, :, :]
        attn = jax.nn.softmax(attn, axis=-1)
        o = jnp.einsum("whnm,whmd->wnhd", attn, v).reshape(NWIN, GG, DIM)
        return o @ proj_w.T + proj_b                     # (NWIN, GG, C)

    fn = jax.pmap(core_fn, in_axes=(0, None, None, None, None, None, None),
                  devices=devs)
    _jit_cache["fn"] = fn
    return fn


def _numpy_fallback(xw, qkv_w, qkv_b, proj_w, proj_b, rpb, mask):
    out = np.empty((B, NWIN, GG, DIM), np.float32)
    for b in range(B):
        qkv = xw[b].reshape(-1, DIM) @ qkv_w.T + qkv_b
        qkv = qkv.reshape(NWIN, GG, 3, NH, HD).transpose(2, 0, 3, 1, 4)
        q, k, v = qkv[0], qkv[1], qkv[2]
        attn = np.einsum("whnd,whmd->whnm", q * SCALE, k)
        attn += rpb[None]
        attn += mask[:, None, None, :]
        attn -= attn.max(-1, keepdims=True)
        np.exp(attn, out=attn)
        attn /= attn.sum(-1, keepdims=True)
        o = np.einsum("whnm,whmd->wnhd", attn, v).reshape(NWIN, GG, DIM)
        out[b] = o @ proj_w.T + proj_b
    return out


def kernel(x, qkv_w, qkv_b, proj_w, proj_b,
           pos_proj_w, pos_proj_b,
           ln1_g, ln1_b, fc1_w, fc1_b,
           ln2_g, ln2_b, fc2_w, fc2_b,
           ln3_g, ln3_b, fc3_w, fc3_b):
    x = np.asarray(x, np.float32)
    qkv_w = np.asarray(qkv_w, np.float32)
    qkv_b = np.asarray(qkv_b, np.float32)
    proj_w = np.asarray(proj_w, np.float32)
    proj_b = np.asarray(proj_b, np.float32)

    rpb = _pos_bias_np(pos_proj_w, pos_proj_b, ln1_g, ln1_b, fc1_w, fc1_b,
                       ln2_g, ln2_b, fc2_w, fc2_b, ln3_g, ln3_b, fc3_w, fc3_b)
    mask = _attn_mask_np()          # (NWIN, 1, GG)
    xw = _window_partition_np(x)    # (B, NWIN, GG, C)

    ow = _run_device_with_watchdog(xw, qkv_w, qkv_b, proj_w, proj_b, rpb, mask)
    if ow is None:
        ow = _numpy_fallback(xw, qkv_w, qkv_b, proj_w, proj_b, rpb,
                             mask[:, 0, :])
    return _window_merge_np(ow.astype(np.float32))


def _run_device_with_watchdog(xw, qkv_w, qkv_b, proj_w, proj_b, rpb, mask,
                              timeout_s=420.0):
    """Run the pmapped stage; never hang the caller. Returns None on failure.

    The first call includes the one-time neuronx compile (minutes); later
    calls are fast, so the watchdog only shrinks after a first success.
    """
    import threading

    if _jit_cache.get("dead"):
        return None
    if _jit_cache.get("warm"):
        timeout_s = 120.0
    box = {}

    def work():
        try:
            fn = _device_fn()
            box["out"] = np.asarray(
                fn(xw, qkv_w, qkv_b, proj_w, proj_b, rpb, mask))
        except Exception as e:  # compile/execute failure -> numpy fallback
            box["err"] = e

    th = threading.Thread(target=work, daemon=True)
    th.start()
    th.join(timeout_s)
    if th.is_alive() or "out" not in box:
        _jit_cache["dead"] = True
        return None
    _jit_cache["warm"] = True
    return box["out"]

